# revision 50
# baseline (speedup 1.0000x reference)
import sys

sys.path.insert(0, "/opt/trn_rl_repo")

import numpy as np
import ml_dtypes

import concourse.bass as bass
import concourse.bacc as bacc
import concourse.mybir as mybir
import concourse.tile as tile
from concourse.bass_utils import run_bass_kernel_spmd

BF16 = mybir.dt.bfloat16
F32 = mybir.dt.float32
AF = mybir.ActivationFunctionType
ALU = mybir.AluOpType

B, N, CD, GD, NH = 32, 512, 80, 50, 3  # batch, nodes, comp_dim, gat_dim, heads
NC_ = 8            # cores
MPC = B // NC_     # molecules per core = 4
NCH = N // 128     # 128-partition chunks per N = 4
FAH = GD + 1       # head attention cols (wh | ones)
FAO = CD + 1       # out-layer attention cols


def _build_nc():
    nc = bacc.Bacc("TRN2", target_bir_lowering=False, debug=False, num_devices=NC_)

    hT_d = nc.dram_tensor("hT", [CD, MPC, N], BF16, kind="ExternalInput")
    adjm_d = nc.dram_tensor("adjm", [128, MPC, NCH, N], BF16, kind="ExternalInput")
    # heads: [W_h | W_h@a2_h] per head -> [80, NH*(GD+1)]
    wplus_d = nc.dram_tensor("Wplus", [CD, NH * (GD + 1)], BF16, kind="ExternalInput")
    # heads: (W_h@a1_h) replicated across 128 cols -> [80, NH*128]
    w1r_d = nc.dram_tensor("W1R", [CD, NH * 128], BF16, kind="ExternalInput")
    wout_d = nc.dram_tensor("Wout", [GD, NH * (CD + 2)], BF16, kind="ExternalInput")
    id_d = nc.dram_tensor("id128", [128, 128], BF16, kind="ExternalInput")
    out_d = nc.dram_tensor("out", [MPC, FAO, N], BF16, kind="ExternalOutput")

    with tile.TileContext(nc) as tc:
        with (
            nc.allow_low_precision(reason="bf16 transposes; no accumulation"),
            tc.tile_pool(name="persist", bufs=1) as pp,
            tc.tile_pool(name="sb", bufs=3) as sb,
            tc.tile_pool(name="chunk", bufs=4) as cb,
            tc.tile_pool(name="ps", bufs=1, space="PSUM") as ps,
            tc.tile_pool(name="psE", bufs=2, space="PSUM") as psE,
            tc.tile_pool(name="psOT", bufs=2, space="PSUM") as psOT,
            tc.tile_pool(name="headp", bufs=2) as hp,
        ):
            # ---- persistent staging (ordered so molecule 0 starts early) ----
            hT_s = pp.tile([CD, MPC, N], BF16, tag="hT")
            nc.sync.dma_start(hT_s[:, 0], hT_d[:, 0])
            wplus_s = pp.tile([CD, NH * (GD + 1)], BF16, tag="Wplus")
            nc.sync.dma_start(wplus_s[:], wplus_d[:])
            w1r_s = pp.tile([CD, NH * 128], BF16, tag="W1R")
            nc.sync.dma_start(w1r_s[:], w1r_d[:])
            adjm_s = pp.tile([128, MPC, NCH, N], BF16, tag="adjm")
            nc.sync.dma_start(adjm_s[:, 0, 0:2], adjm_d[:, 0, 0:2])
            nc.sync.dma_start(adjm_s[:, 0, 2:4], adjm_d[:, 0, 2:4])
            wout_s = pp.tile([GD, NH * (CD + 2)], BF16, tag="Wout")
            nc.sync.dma_start(wout_s[:], wout_d[:])
            id_s = pp.tile([128, 128], BF16, tag="id")
            nc.sync.dma_start(id_s[:], id_d[:])
            for m in range(1, MPC):
                nc.sync.dma_start(hT_s[:, m], hT_d[:, m])
                nc.sync.dma_start(adjm_s[:, m], adjm_d[:, m])
            onescol_s = pp.tile([1, 128], BF16, tag="onescol")
            nc.vector.memset(onescol_s[:], 1.0)
            # prime the ACT exp table during the DMA shadow
            warmt = pp.tile([1, 2], BF16, tag="warmt")
            nc.vector.memset(warmt[:], 0.0)
            nc.scalar.activation(warmt[:], warmt[:], AF.Exp)

            # out layer: rows 0:2 e-rows, 2:82 whT_out, 82 ones (persistent)
            whsX = pp.tile([83, N], BF16, tag="whsX")
            nc.vector.memset(whsX[:], 1.0)
            # wha per head: col GD is ones (persistent); cols 0:GD rewritten
            whaH = []
            for h in range(NH):
                wt = pp.tile([128, NCH, FAH + 1], BF16, tag=f"whaH{h}")
                nc.vector.memset(wt[:, :, GD:FAH], 1.0)
                whaH.append(wt)

            def prep_head(m, h):
                """Wh/e-term matmuls + PSUM->SBUF staging for head layer."""
                whnf = ps.tile([128, NCH, FAO], F32, tag="whn")
                whn = whnf[:, :, 0 : GD + 1]
                for c in range(NCH):
                    nc.tensor.matmul(
                        whn[:, c, :],
                        hT_s[:, m, c * 128 : (c + 1) * 128],
                        wplus_s[:, h * (GD + 1) : (h + 1) * (GD + 1)],
                        start=True, stop=True,
                    )
                E1p = psE.tile([128, N], F32, tag="E1p")
                nc.tensor.matmul(
                    E1p[:], w1r_s[:, h * 128 : (h + 1) * 128],
                    hT_s[:, m, :], start=True, stop=True,
                )
                wha = whaH[h]
                nc.scalar.activation(wha[:, :, 0:GD], whn[:, :, 0:GD], AF.Copy)
                e2cs = sb.tile([128, NCH], F32, tag="e2cs")
                nc.vector.tensor_copy(e2cs[:], whn[:, :, GD])
                e2cm = sb.tile([128, NCH], F32, tag="e2cm")
                nc.vector.tensor_scalar_mul(e2cm[:], e2cs[:], -0.8)
                E1b = sb.tile([128, N], BF16, tag="E1b")
                nc.scalar.activation(E1b[:], E1p[:], AF.Copy)
                return dict(m=m, Fo=GD, FA=FAH, wha=wha, e2cs=e2cs,
                            e2cm=e2cm, E1b=E1b, tagp="H")

            def prep_out(m, headTs_list):
                # fused stationary [aoW_h | wout_h]: rows 0:2 = e-rows,
                # rows 2:82 = whT_out in one matmul stream
                whpO = ps.tile([82, N], F32, tag="whpO")
                for h in range(NH):
                    nc.tensor.matmul(
                        whpO[:], wout_s[:, h * (CD + 2) : (h + 1) * (CD + 2)],
                        headTs_list[h][:], start=(h == 0), stop=(h == NH - 1),
                    )
                nc.scalar.activation(whsX[0:82, :], whpO[:], AF.Copy)
                # one transpose set: cols = [e1, e2, wh x80, ones]
                xp = ps.tile([128, NCH, 256], BF16, tag="xp")
                whnO = xp[:, :, 96 : 96 + 83]
                for c in range(NCH):
                    nc.tensor.transpose(
                        whnO[:, c, :], whsX[:, c * 128 : (c + 1) * 128],
                        id_s[0:83, 0:83],
                    )
                e2csO = sb.tile([128, NCH], F32, tag="e2csO")
                nc.vector.tensor_copy(e2csO[:], whnO[:, :, 1])
                e2cmO = sb.tile([128, NCH], F32, tag="e2cmO")
                nc.vector.tensor_scalar_mul(e2cmO[:], e2csO[:], -0.8)
                whaOf = sb.tile([128, NCH, 84], BF16, tag="whaO")
                nc.scalar.activation(whaOf[:, :, 0:83], whnO[:], AF.Copy)
                whaO = whaOf[:, :, 2:83]
                E1pO = psE.tile([128, N], F32, tag="E1p")
                nc.tensor.matmul(
                    E1pO[:], onescol_s[:], whsX[0:1, :], start=True, stop=True
                )
                E1bO = sb.tile([128, N], BF16, tag="E1bO")
                nc.scalar.activation(E1bO[:], E1pO[:], AF.Copy)
                return dict(m=m, Fo=CD, FA=FAO, wha=whaO, e2cs=e2csO,
                            e2cm=e2cmO, E1b=E1bO, tagp="O")

            def attention(P, filler=None):
                """Masked GAT attention. exp(lrelu(e)) = exp(max(A,
                0.2A - 0.8*e2) + e2) with A = e1 + adjm (mask additive).
                Returns Ysb [128, NCH, FA] bf16 (num | den) and R."""
                m, FA, Fo = P["m"], P["FA"], P["Fo"]
                wha, e2cs, e2cm, E1b = P["wha"], P["e2cs"], P["e2cm"], P["E1b"]
                OTf = psOT.tile([FAO, N], F32, tag="OT")
                OT = OTf[0:FA, :]
                # chunk groups [0], [1,2], [3]: first exp starts early,
                # middle pair amortizes the DVE init bubble
                for gi, grp in enumerate(((0,), (1, 2), (3,))):
                    w = len(grp)
                    c0 = grp[0]
                    E1bx = E1b[:].unsqueeze(1).broadcast_to((128, w, N))
                    Ap = cb.tile([128, w, N], BF16, tag=f"A{gi}")
                    nc.vector.tensor_tensor(
                        Ap[:], E1bx, adjm_s[:, m, c0 : c0 + w, :], op=ALU.add
                    )
                    Bp = cb.tile([128, w, N], BF16, tag=f"B{gi}")
                    for j in range(w):
                        nc.vector.tensor_scalar(
                            Bp[:, j, :], Ap[:, j, :], 0.2,
                            e2cm[:, c0 + j : c0 + j + 1],
                            op0=ALU.mult, op1=ALU.add,
                        )
                    Mp = cb.tile([128, w, N], BF16, tag=f"M{gi}")
                    nc.vector.tensor_tensor(Mp[:], Ap[:], Bp[:], op=ALU.max)
                    for j in range(w):
                        c = c0 + j
                        EA = cb.tile([128, N], BF16, tag=f"EA{c}")
                        nc.scalar.activation(
                            EA[:], Mp[:, j, :], AF.Exp, bias=e2cs[:, c : c + 1]
                        )
                        nc.tensor.matmul(
                            OT[:], wha[:, c, 0:FA], EA[:],
                            start=(c == 0), stop=(c == NCH - 1),
                        )
                    if gi == 0 and filler is not None:
                        filler()
                OTs = sb.tile([FA, N], BF16, tag="OTs" + P["tagp"])
                if P["tagp"] == "O":
                    nc.vector.tensor_copy(OTs[:], OT[:])
                    return OTs, None  # raw (num | den) rows; host normalizes
                nc.scalar.activation(OTs[:], OT[:], AF.Copy)
                xp = ps.tile([128, NCH, 256], BF16, tag="xp")
                TOT = xp[:, :, 0:FA]
                for c in range(NCH):
                    nc.tensor.transpose(
                        TOT[:, c, :], OTs[:, c * 128 : (c + 1) * 128],
                        id_s[0:FA, 0:FA],
                    )
                Ysbf = sb.tile([128, NCH, FA + 1], BF16, tag="Ysb" + P["tagp"])
                Ysb = Ysbf[:, :, 0:FA]
                nc.vector.tensor_copy(Ysb[:], TOT[:])
                R = sb.tile([128, NCH], F32, tag="R" + P["tagp"])
                nc.vector.reciprocal(R[:], Ysb[:, :, Fo])
                return Ysb, R

            def post_head(m, h, Ysb, R):
                """normalize + ELU + row-layout transpose for a head layer"""
                Y = sb.tile([128, NCH, GD], BF16, tag="Yh")
                for c in range(NCH):
                    nc.vector.tensor_scalar_mul(
                        Y[:, c, :], Ysb[:, c, 0:GD], R[:, c : c + 1]
                    )
                # ELU(y) = max(y, min(exp(y), 1) - 1)
                EX = sb.tile([128, NCH, GD], BF16, tag="EX")
                nc.scalar.activation(EX[:], Y[:], AF.Exp)
                nc.vector.tensor_scalar(
                    EX[:], EX[:], 1.0, -1.0, op0=ALU.min, op1=ALU.add
                )
                EL = sb.tile([128, NCH, GD], BF16, tag="EL")
                nc.vector.tensor_tensor(EL[:], Y[:], EX[:], op=ALU.max)
                # transpose to row layout for the out layer
                hTp = ps.tile([GD, NCH, 128], BF16, tag="headT")
                for c in range(NCH):
                    nc.tensor.transpose(
                        hTp[:, c, :], EL[:, c, :], id_s[0:128, 0:128]
                    )
                hts = hp.tile([GD, NCH, 128], BF16, tag=f"headTs{h}")
                if (m * NH + h) % 4 != 1:
                    nc.vector.tensor_copy(hts[:], hTp[:])
                else:
                    nc.scalar.activation(hts[:], hTp[:], AF.Copy)
                return hts.rearrange("o c p -> o (c p)")

            def do_out(m, P):
                OTs, _ = attention(P)
                nc.sync.dma_start(out_d[m], OTs[:])

            # software-pipelined: prep for the next head layer is emitted
            # before the current layer's attention consumes the engines
            preps = {(0, 0): prep_head(0, 0)}
            pending_out = None
            for m in range(MPC):
                headTs_list = []
                pending_post = None
                for h in range(NH):
                    P = preps.pop((m, h))
                    nxt = (m, h + 1) if h < NH - 1 else (m + 1, 0)

                    def filler(nxt=nxt):
                        if nxt[0] < MPC and nxt not in preps:
                            preps[nxt] = prep_head(*nxt)

                    Ysb, R = attention(P, filler)
                    # post-processing of the previous head layer hides
                    # behind this attention's engine work
                    if pending_post is not None:
                        headTs_list.append(post_head(*pending_post))
                    pending_post = (m, h, Ysb, R)
                    # out-layer of the previous molecule runs concurrent
                    # with this molecule's second head attention; the last
                    # two out-layers interleave each other at the end
                    flush_at = 1 if m < MPC - 1 else 99
                    if h == flush_at and pending_out is not None:
                        do_out(*pending_out)
                        pending_out = None
                headTs_list.append(post_head(*pending_post))
                PO = prep_out(m, headTs_list)
                if pending_out is not None:
                    do_out(*pending_out)
                    pending_out = None
                pending_out = (m, PO)
            do_out(*pending_out)

    nc.compile()
    return nc


_NC_CACHE = None
_LAST_IN_MAPS = None


def build_nc():
    global _NC_CACHE
    if _NC_CACHE is None:
        _NC_CACHE = _build_nc()
    return _NC_CACHE


def assemble_core_output(parts):
    """Map one core's output tensors -> [MPC, N, CD] float32.

    Device ships raw attention rows [MPC, FAO, N]: rows 0:CD are the
    unnormalized outputs, row CD the softmax denominators."""
    raw = np.asarray(parts["out"], dtype=np.float32)
    return (raw[:, 0:CD, :] / raw[:, CD : CD + 1, :]).transpose(0, 2, 1)


def prep_in_maps(h, adj, Ws, attn_a, W_out, a_out):
    h = np.asarray(h, dtype=np.float32)
    adj = np.asarray(adj)
    Ws = np.asarray(Ws, dtype=np.float32)
    attn_a = np.asarray(attn_a, dtype=np.float32)
    W_out = np.asarray(W_out, dtype=np.float32)
    a_out = np.asarray(a_out, dtype=np.float32)
    bf16 = ml_dtypes.bfloat16

    # heads: Wplus = [W_h | W_h@a2_h], W1R = (W_h@a1_h) replicated
    wplus = np.zeros((CD, NH * (GD + 1)), np.float32)
    w1r = np.zeros((CD, NH * 128), np.float32)
    for hh in range(NH):
        wplus[:, hh * (GD + 1) : hh * (GD + 1) + GD] = Ws[hh]
        wplus[:, hh * (GD + 1) + GD] = Ws[hh] @ attn_a[hh, GD:]
        w1r[:, hh * 128 : (hh + 1) * 128] = (Ws[hh] @ attn_a[hh, :GD])[:, None]
    # out layer: per-head stationary [aoW_h | wout_h] where
    # aoW_h = W_out_block @ (a_out[:CD], a_out[CD:]) gives the e-rows
    ao = np.stack([a_out[:CD], a_out[CD:]], axis=1)  # [CD, 2]
    wout_b = np.zeros((GD, NH * (CD + 2)), np.float32)
    for hh in range(NH):
        blk = W_out[hh * GD : (hh + 1) * GD, :]  # [GD, CD]
        wout_b[:, hh * (CD + 2) : hh * (CD + 2) + 2] = blk @ ao
        wout_b[:, hh * (CD + 2) + 2 : (hh + 1) * (CD + 2)] = blk
    id128 = np.eye(128, dtype=np.float32)


    in_maps = []
    for k in range(NC_):
        mols = slice(k * MPC, (k + 1) * MPC)
        hT_core = np.ascontiguousarray(h[mols].transpose(2, 0, 1))  # [80,4,512]
        a = adj[mols].astype(np.float32).transpose(0, 2, 1)  # [4, j, i]
        a = (a - 1.0) * 256.0  # additive mask: 0 kept, -256 masked
        a = a.reshape(MPC, NCH, 128, N).transpose(2, 0, 1, 3)  # [128,4,4,512]
        in_maps.append(
            {
                "hT": hT_core.astype(bf16),
                "adjm": np.ascontiguousarray(a).astype(bf16),
                "Wplus": wplus.astype(bf16),
                "W1R": w1r.astype(bf16),
                "Wout": wout_b.astype(bf16),
                "id128": id128.astype(bf16),
            }
        )
    return in_maps


def kernel(h, adj, Ws, attn_a, W_out, a_out):
    in_maps = prep_in_maps(h, adj, Ws, attn_a, W_out, a_out)
    global _LAST_IN_MAPS
    _LAST_IN_MAPS = in_maps
    nc = build_nc()
    res = run_bass_kernel_spmd(nc, in_maps, core_ids=list(range(NC_)))
    outs = [assemble_core_output(res.results[k]) for k in range(NC_)]
    return np.concatenate(outs, axis=0).reshape(B, N, CD)


if __name__ == "__main__":
    import reference

    inputs = {k: np.asarray(v) for k, v in reference.setup_inputs().items()}
    exp = np.asarray(reference.reference(**inputs))
    got = kernel(**inputs)
    err = np.abs(got - exp).max() / (np.abs(exp).max() + 1e-9)
    print("Relative error:", err)
